# revision 8
# baseline (speedup 1.0000x reference)
"""BinaryLinear TRN2 kernel.

Computes out = inputs @ (sign(W) * scale).T + bias where
  sign(w) = +1 for w >= 0 else -1
  scale[o] = max(mean_i |W[o, i]|, 1e-6)

Problem shapes (hardcoded): inputs [8192, 4096] f32, weight [4096, 4096] f32,
bias [4096] f32 -> out [8192, 4096] f32.

Distribution: data-parallel over tokens. Each of the 8 cores gets a
[1024, 4096] slice of `inputs` and the full weight/bias, and produces a
[1024, 4096] slice of the output. No collectives; host concatenates.

Per-core algorithm (v2 - PE does matmul ONLY, transposes on the DMA XBAR):
  - X^T: DMA X f32 chunks, DVE-cast to bf16, then dma_start_transpose
    (XBAR, 16x128 tiles) into resident X^T [128, 32, 1024] bf16.
  - Weights: stream W by 128-row tiles; ACT computes Sign into bf16
    (exact +-1), DVE reduces |W| row sums (scale), dma_start_transpose
    builds S^T [128, 32, 512] per 512-col output chunk. No PE transposes.
  - Matmul: psum[t128, o512] accumulates over 32 k-tiles,
    lhsT = X^T tile (stationary), rhs = S^T slice (moving). 2048 MMs of
    N=512 == the bf16 roofline (~437us/core); everything else overlaps.
  - Evict: out = psum * scale[o] + bias[o] on DVE (broadcast rows via
    stride-0 DMA), out-DMA dispatched from gpsimd (own queue).

Queues: sync ring = ALL XBAR transposes (exclusively - concurrent XBAR
transposes from both HWDGE rings corrupt data); scalar ring = X/W loads +
scale/bias broadcasts; gpsimd = X loads + scale scratch + output stores.

Only X's bf16 rounding introduces error (~1.7e-3 relative).
"""

import os
import sys

import numpy as np

sys.path.insert(0, "/opt/trn_rl_repo")

import concourse.bass as bass
import concourse.mybir as mybir
from concourse import bacc
import concourse.tile as tile


def _ensure_ntff_hook():
    """The agent image's `antenv` lacks `axon_hooks`, which
    run_bass_kernel_spmd imports when trace=True (for HW exec timing).
    Provide the module and install the standard ctypes-based hook.
    Harmless when tracing is off (the import never fires)."""
    import types

    try:
        import antenv.axon_hooks  # noqa: F401
        return
    except ImportError:
        pass
    try:
        import antenv
    except ImportError:
        return
    mod = types.ModuleType("antenv.axon_hooks")
    state = {"hook": None}
    mod.set_axon_ntff_profile_hook = lambda h: state.update(hook=h)
    mod.get_axon_ntff_profile_hook = lambda: state["hook"]
    sys.modules["antenv.axon_hooks"] = mod
    antenv.axon_hooks = mod
    try:
        from trn_agent_boot.trn_boot import _ntff_profile_via_ctypes

        hook = _ntff_profile_via_ctypes("/opt/axon/libaxon_pjrt.so")
        if hook is not None:
            mod.set_axon_ntff_profile_hook(hook)
    except Exception:
        pass


_ensure_ntff_hook()

F32 = mybir.dt.float32
BF16 = mybir.dt.bfloat16

TOKENS = 8192
IN_FEATURES = 4096
OUT_FEATURES = 4096
N_CORES = 8


def build_nc(t_core, in_f, out_f, och=512):
    """Build the per-core Bass module. All cores run the identical program."""
    P = 128
    CH = 2048                      # f32 staging chunk (8KB per partition)
    t_tiles = t_core // P          # 8 token tiles
    i_tiles = in_f // P            # 32 contraction tiles
    n_ch = in_f // CH              # 2 chunks per 128-row tile
    kt_ch = CH // P                # 16 k-tiles per chunk
    oc_chunks = out_f // och       # 8 output column chunks
    o_tiles = och // P             # 4 W row-tiles per chunk

    nc = bacc.Bacc()
    x_dram = nc.dram_tensor("x", [t_core, in_f], F32, kind="ExternalInput")
    w_dram = nc.dram_tensor("w", [out_f, in_f], F32, kind="ExternalInput")
    b_dram = nc.dram_tensor("b", [out_f], F32, kind="ExternalInput")
    out_dram = nc.dram_tensor("out", [t_core, out_f], F32, kind="ExternalOutput")

    with tile.TileContext(nc) as tc:
        with (
            tc.tile_pool(name="const", bufs=1) as const,
            tc.tile_pool(name="resident", bufs=1) as resident,
            tc.tile_pool(name="st", bufs=2) as st_pool,
            tc.tile_pool(name="xf32", bufs=2) as xf32,
            tc.tile_pool(name="wf32", bufs=2) as wf32,
            tc.tile_pool(name="xb16", bufs=2) as xb16,
            tc.tile_pool(name="wb16", bufs=2) as wb16,
            tc.tile_pool(name="small", bufs=8) as small,
            tc.tile_pool(name="scbc", bufs=3) as scbc,
            tc.tile_pool(name="outsb", bufs=4) as outsb,
            tc.tile_pool(name="psum_mm", bufs=6, space="PSUM") as psum_mm,
            tc.tile_pool(name="psum_warm", bufs=1, space="PSUM") as psum_warm,
            tc.tile_pool(name="dram", bufs=1, space="DRAM") as dram_pool,
        ):
            # tiny positive bias so Sign(0 + tiny) = +1, matching the
            # reference's where(w >= 0, 1, -1)
            signbias = const.tile([P, 1], F32)
            nc.vector.memset(signbias[:], 1e-30)

            # per-row scale scratch in DRAM (written column-major by o-tile,
            # read back with a partition-broadcast AP)
            scale_dram = dram_pool.tile([out_f], F32)
            scale_pm = scale_dram[:].rearrange("(g p) -> p g", p=P)

            # resident X^T: xt[p, ktile, t] = X[t, ktile*128 + p]
            xt = resident.tile([P, i_tiles, t_core], BF16)

            warm_count = [0]

            def emit_warm(src16):
                """A tiny matmul tied to freshly-cast data keeps the PE
                activity monitor (HAM) from re-throttling during the
                DMA/ACT-heavy prologue. Result is discarded."""
                if warm_count[0] >= 24:
                    return
                warm_count[0] += 1
                wp = psum_warm.tile([P, P], F32, tag="warm")
                nc.tensor.matmul(
                    wp[:], src16[:, :P], src16[:, :P], start=True, stop=True
                )

            def emit_x_load(t):
                chunks = []
                for c in range(n_ch):
                    xs = xf32.tile([P, CH], F32, tag="xstage")
                    nc.gpsimd.dma_start(
                        xs[:], x_dram[t * P:(t + 1) * P, c * CH:(c + 1) * CH]
                    )
                    chunks.append(xs)
                return chunks

            def emit_x_cast_tr(t, chunks):
                for c in range(n_ch):
                    xb = xb16.tile([P, CH], BF16, tag="xbstage")
                    nc.vector.tensor_copy(xb[:], chunks[c][:])
                    nc.sync.dma_start_transpose(
                        xt[:, c * kt_ch:(c + 1) * kt_ch, t * P:(t + 1) * P],
                        xb[:],
                    )
                    emit_warm(xb)

            def emit_w_block(oc):
                """Stream W rows for one 512-col output chunk: sign -> S^T
                via XBAR transpose, |W| row sums -> scale, plus broadcast
                rows for scale and bias."""
                st = st_pool.tile([P, i_tiles, och], BF16, tag="st")
                scale_cols = small.tile([P, o_tiles], F32, tag="scale_cols")
                for ot in range(o_tiles):
                    o_row = (oc * o_tiles + ot) * P
                    red = small.tile([P, n_ch], F32, tag="red")
                    for c in range(n_ch):
                        ws = wf32.tile([P, CH], F32, tag="wstage")
                        nc.scalar.dma_start(
                            ws[:], w_dram[o_row:o_row + P, c * CH:(c + 1) * CH]
                        )
                        sn = wb16.tile([P, CH], BF16, tag="wbstage")
                        nc.scalar.activation(
                            sn[:], ws[:], mybir.ActivationFunctionType.Sign,
                            bias=signbias[:],
                        )
                        nc.vector.tensor_reduce(
                            red[:, c:c + 1], ws[:],
                            axis=mybir.AxisListType.X, op=mybir.AluOpType.add,
                            apply_absolute_value=True,
                        )
                        if oc < 2:
                            emit_warm(sn)
                        nc.sync.dma_start_transpose(
                            st[:, c * kt_ch:(c + 1) * kt_ch,
                               ot * P:(ot + 1) * P],
                            sn[:],
                        )
                    redt = small.tile([P, 1], F32, tag="redt")
                    nc.vector.tensor_reduce(
                        redt[:], red[:],
                        axis=mybir.AxisListType.X, op=mybir.AluOpType.add,
                    )
                    nc.vector.tensor_scalar(
                        scale_cols[:, ot:ot + 1], redt[:],
                        1.0 / in_f, 1e-6,
                        op0=mybir.AluOpType.mult, op1=mybir.AluOpType.max,
                    )
                nc.gpsimd.dma_start(
                    scale_pm[:, oc * o_tiles:(oc + 1) * o_tiles], scale_cols[:]
                )
                sc_bc = scbc.tile([P, och], F32, tag="scbc")
                sc_slice = scale_dram[oc * och:(oc + 1) * och]
                nc.scalar.dma_start(
                    sc_bc[:],
                    bass.AP(tensor=sc_slice.tensor, offset=sc_slice.offset,
                            ap=[[0, P]] + list(sc_slice.ap)),
                )
                bias_bc = scbc.tile([P, och], F32, tag="biasbc")
                b_slice = b_dram[oc * och:(oc + 1) * och]
                nc.scalar.dma_start(
                    bias_bc[:],
                    bass.AP(tensor=b_slice.tensor, offset=b_slice.offset,
                            ap=[[0, P]] + list(b_slice.ap)),
                )
                return st, sc_bc, bias_bc

            def emit_mm_block(oc, t, st, sc_bc, bias_bc):
                pm = psum_mm.tile([P, och], F32, tag="mmps")
                for i in range(i_tiles):
                    nc.tensor.matmul(
                        pm[:],
                        xt[:, i, t * P:(t + 1) * P],
                        st[:, i, :],
                        start=(i == 0), stop=(i == i_tiles - 1),
                    )
                ob = outsb.tile([P, och], F32, tag="ob")
                nc.vector.tensor_mul(out=ob[:], in0=pm[:], in1=sc_bc[:])
                nc.vector.tensor_add(out=ob[:], in0=ob[:], in1=bias_bc[:])
                nc.gpsimd.dma_start(
                    out_dram[t * P:(t + 1) * P, oc * och:(oc + 1) * och],
                    ob[:],
                )

            # ---- emission order: prologue interleaves X build with W(oc0/1)
            # so the first MM fires ~25-40us in and the PE never starves.
            xl = {}
            xl[0] = emit_x_load(0)
            xl[1] = emit_x_load(1)
            xl[2] = emit_x_load(2)
            emit_x_cast_tr(0, xl[0])
            wctx = {}
            wctx[0] = emit_w_block(0)
            emit_x_cast_tr(1, xl[1])
            xl[3] = emit_x_load(3)
            xl[4] = emit_x_load(4)
            emit_x_cast_tr(2, xl[2])
            emit_x_cast_tr(3, xl[3])
            xl[5] = emit_x_load(5)
            wctx[1] = emit_w_block(1)
            emit_x_cast_tr(4, xl[4])
            xl[6] = emit_x_load(6)
            emit_x_cast_tr(5, xl[5])
            xl[7] = emit_x_load(7)
            emit_x_cast_tr(6, xl[6])
            emit_x_cast_tr(7, xl[7])

            for oc in range(oc_chunks):
                for t in range(t_tiles):
                    emit_mm_block(oc, t, *wctx[oc])
                if oc + 2 < oc_chunks:
                    wctx[oc + 2] = emit_w_block(oc + 2)
                del wctx[oc]

    nc.finalize()
    return nc


_CACHE = {}


def kernel(inputs, weight, bias):
    from concourse.bass_utils import run_bass_kernel_spmd

    x = np.ascontiguousarray(np.asarray(inputs, dtype=np.float32))
    w = np.ascontiguousarray(np.asarray(weight, dtype=np.float32))
    b = np.ascontiguousarray(np.asarray(bias, dtype=np.float32))
    assert x.shape == (TOKENS, IN_FEATURES)
    assert w.shape == (OUT_FEATURES, IN_FEATURES)
    assert b.shape == (OUT_FEATURES,)

    if "nc" not in _CACHE:
        _CACHE["nc"] = build_nc(TOKENS // N_CORES, IN_FEATURES, OUT_FEATURES)
    nc = _CACHE["nc"]

    shards = np.split(x, N_CORES, axis=0)
    in_maps = [{"x": shards[c], "w": w, "b": b} for c in range(N_CORES)]
    trace = bool(os.environ.get("BASS_TRACE"))
    res = run_bass_kernel_spmd(nc, in_maps, list(range(N_CORES)), trace=trace)
    if trace:
        _CACHE["last_result"] = res
        if res.exec_time_ns is not None:
            print(f"HW exec time: {res.exec_time_ns} ns")

    return np.concatenate([res.results[c]["out"] for c in range(N_CORES)], axis=0)


# revision 9
# speedup vs baseline: 1.2948x; 1.2948x over previous
"""BinaryLinear TRN2 kernel.

Computes out = inputs @ (sign(W) * scale).T + bias where
  sign(w) = +1 for w >= 0 else -1
  scale[o] = max(mean_i |W[o, i]|, 1e-6)

Problem shapes (hardcoded): inputs [8192, 4096] f32, weight [4096, 4096] f32,
bias [4096] f32 -> out [8192, 4096] f32.

Distribution: data-parallel over tokens. Each of the 8 cores gets a
[1024, 4096] slice of `inputs` and the full weight/bias, and produces a
[1024, 4096] slice of the output. No collectives; host concatenates.

Per-core algorithm (v2 - PE does matmul ONLY, transposes on the DMA XBAR):
  - X^T: DMA X f32 chunks, DVE-cast to bf16, then dma_start_transpose
    (XBAR, 16x128 tiles) into resident X^T [128, 32, 1024] bf16.
  - Weights: stream W by 128-row tiles; ACT computes Sign into bf16
    (exact +-1), DVE reduces |W| row sums (scale), dma_start_transpose
    builds S^T [128, 32, 512] per 512-col output chunk. No PE transposes.
  - Matmul: psum[t128, o512] accumulates over 32 k-tiles,
    lhsT = X^T tile (stationary), rhs = S^T slice (moving). 2048 MMs of
    N=512 == the bf16 roofline (~437us/core); everything else overlaps.
  - Evict: out = psum * scale[o] + bias[o] on DVE (broadcast rows via
    stride-0 DMA), out-DMA dispatched from gpsimd (own queue).

S^T is built on the PE (transpose mode) + DVE psum evict: the S^T supply
chain must use only engine semaphores - routing it through DMA instructions
(XBAR) exposes it to bacc's DMA-semaphore sharing, which serializes the
transposes behind unrelated loads/stores (7-24us stalls in front of the PE).
X^T uses the XBAR DMA transpose, but only in the prologue where slack covers
those waits. Concurrent XBAR transposes from both HWDGE rings corrupt data -
all XBAR transposes stay on the sync ring. scalar ring = W loads +
scale/bias broadcasts; gpsimd = X loads + scale scratch + output stores.

Only X's bf16 rounding introduces error (~1.7e-3 relative).
"""

import os
import sys

import numpy as np

sys.path.insert(0, "/opt/trn_rl_repo")

import concourse.bass as bass
import concourse.mybir as mybir
from concourse import bacc
import concourse.tile as tile
from concourse.masks import make_identity


def _ensure_ntff_hook():
    """The agent image's `antenv` lacks `axon_hooks`, which
    run_bass_kernel_spmd imports when trace=True (for HW exec timing).
    Provide the module and install the standard ctypes-based hook.
    Harmless when tracing is off (the import never fires)."""
    import types

    try:
        import antenv.axon_hooks  # noqa: F401
        return
    except ImportError:
        pass
    try:
        import antenv
    except ImportError:
        return
    mod = types.ModuleType("antenv.axon_hooks")
    state = {"hook": None}
    mod.set_axon_ntff_profile_hook = lambda h: state.update(hook=h)
    mod.get_axon_ntff_profile_hook = lambda: state["hook"]
    sys.modules["antenv.axon_hooks"] = mod
    antenv.axon_hooks = mod
    try:
        from trn_agent_boot.trn_boot import _ntff_profile_via_ctypes

        hook = _ntff_profile_via_ctypes("/opt/axon/libaxon_pjrt.so")
        if hook is not None:
            mod.set_axon_ntff_profile_hook(hook)
    except Exception:
        pass


_ensure_ntff_hook()

F32 = mybir.dt.float32
BF16 = mybir.dt.bfloat16

TOKENS = 8192
IN_FEATURES = 4096
OUT_FEATURES = 4096
N_CORES = 8


def build_nc(t_core, in_f, out_f, och=512):
    """Build the per-core Bass module. All cores run the identical program."""
    P = 128
    CH = 2048                      # f32 staging chunk (8KB per partition)
    t_tiles = t_core // P          # 8 token tiles
    i_tiles = in_f // P            # 32 contraction tiles
    n_ch = in_f // CH              # 2 chunks per 128-row tile
    kt_ch = CH // P                # 16 k-tiles per chunk
    oc_chunks = out_f // och       # 8 output column chunks
    o_tiles = och // P             # 4 W row-tiles per chunk

    nc = bacc.Bacc()
    x_dram = nc.dram_tensor("x", [t_core, in_f], F32, kind="ExternalInput")
    w_dram = nc.dram_tensor("w", [out_f, in_f], F32, kind="ExternalInput")
    b_dram = nc.dram_tensor("b", [out_f], F32, kind="ExternalInput")
    out_dram = nc.dram_tensor("out", [t_core, out_f], F32, kind="ExternalOutput")

    with tile.TileContext(nc) as tc:
        with (
            tc.tile_pool(name="const", bufs=1) as const,
            tc.tile_pool(name="resident", bufs=1) as resident,
            tc.tile_pool(name="st", bufs=2) as st_pool,
            tc.tile_pool(name="xf32", bufs=2) as xf32,
            tc.tile_pool(name="wf32", bufs=2) as wf32,
            tc.tile_pool(name="xb16", bufs=2) as xb16,
            tc.tile_pool(name="wb16", bufs=2) as wb16,
            tc.tile_pool(name="small", bufs=8) as small,
            tc.tile_pool(name="scbc", bufs=3) as scbc,
            tc.tile_pool(name="outsb", bufs=4) as outsb,
            tc.tile_pool(name="psum_mm", bufs=5, space="PSUM") as psum_mm,
            tc.tile_pool(name="psum_tr", bufs=2, space="PSUM") as psum_tr,
            tc.tile_pool(name="psum_warm", bufs=1, space="PSUM") as psum_warm,
            tc.tile_pool(name="dram", bufs=1, space="DRAM") as dram_pool,
        ):
            # tiny positive bias so Sign(0 + tiny) = +1, matching the
            # reference's where(w >= 0, 1, -1)
            signbias = const.tile([P, 1], F32)
            nc.vector.memset(signbias[:], 1e-30)
            ident = const.tile([P, P], BF16)
            make_identity(nc, ident)

            # per-row scale scratch in DRAM (written column-major by o-tile,
            # read back with a partition-broadcast AP)
            scale_dram = dram_pool.tile([out_f], F32)
            scale_pm = scale_dram[:].rearrange("(g p) -> p g", p=P)

            # resident X^T: xt[p, ktile, t] = X[t, ktile*128 + p]
            xt = resident.tile([P, i_tiles, t_core], BF16)

            warm_count = [0]

            def emit_warm(src16):
                """A tiny matmul tied to freshly-cast data keeps the PE
                activity monitor (HAM) from re-throttling during the
                DMA/ACT-heavy prologue. Result is discarded."""
                if warm_count[0] >= 24:
                    return
                warm_count[0] += 1
                wp = psum_warm.tile([P, P], F32, tag="warm")
                nc.tensor.matmul(
                    wp[:], src16[:, :P], src16[:, :P], start=True, stop=True
                )

            def emit_x_load(t):
                chunks = []
                for c in range(n_ch):
                    xs = xf32.tile([P, CH], F32, tag="xstage")
                    nc.gpsimd.dma_start(
                        xs[:], x_dram[t * P:(t + 1) * P, c * CH:(c + 1) * CH]
                    )
                    chunks.append(xs)
                return chunks

            def emit_x_cast_tr(t, chunks):
                for c in range(n_ch):
                    xb = xb16.tile([P, CH], BF16, tag="xbstage")
                    nc.vector.tensor_copy(xb[:], chunks[c][:])
                    nc.sync.dma_start_transpose(
                        xt[:, c * kt_ch:(c + 1) * kt_ch, t * P:(t + 1) * P],
                        xb[:],
                    )
                    emit_warm(xb)

            def emit_w_block(oc):
                """Stream W rows for one 512-col output chunk: sign -> S^T
                via XBAR transpose, |W| row sums -> scale, plus broadcast
                rows for scale and bias."""
                st = st_pool.tile([P, i_tiles, och], BF16, tag="st")
                scale_cols = small.tile([P, o_tiles], F32, tag="scale_cols")
                for ot in range(o_tiles):
                    o_row = (oc * o_tiles + ot) * P
                    red = small.tile([P, n_ch], F32, tag="red")
                    for c in range(n_ch):
                        ws = wf32.tile([P, CH], F32, tag="wstage")
                        nc.scalar.dma_start(
                            ws[:], w_dram[o_row:o_row + P, c * CH:(c + 1) * CH]
                        )
                        sn = wb16.tile([P, CH], BF16, tag="wbstage")
                        nc.scalar.activation(
                            sn[:], ws[:], mybir.ActivationFunctionType.Sign,
                            bias=signbias[:],
                        )
                        nc.vector.tensor_reduce(
                            red[:, c:c + 1], ws[:],
                            axis=mybir.AxisListType.X, op=mybir.AluOpType.add,
                            apply_absolute_value=True,
                        )
                        if oc < 2:
                            emit_warm(sn)
                        for h in range(2):
                            ps = psum_tr.tile([P, 1024], BF16, tag="trps")
                            for j in range(8):
                                nc.tensor.transpose(
                                    ps[:, j * P:(j + 1) * P],
                                    sn[:, h * 1024 + j * P:h * 1024 + (j + 1) * P],
                                    ident[:],
                                )
                            k0 = c * kt_ch + h * 8
                            nc.vector.tensor_copy(
                                st[:, k0:k0 + 8, ot * P:(ot + 1) * P],
                                ps[:].rearrange("p (s q) -> p s q", q=P),
                            )
                    redt = small.tile([P, 1], F32, tag="redt")
                    nc.vector.tensor_reduce(
                        redt[:], red[:],
                        axis=mybir.AxisListType.X, op=mybir.AluOpType.add,
                    )
                    nc.vector.tensor_scalar(
                        scale_cols[:, ot:ot + 1], redt[:],
                        1.0 / in_f, 1e-6,
                        op0=mybir.AluOpType.mult, op1=mybir.AluOpType.max,
                    )
                nc.gpsimd.dma_start(
                    scale_pm[:, oc * o_tiles:(oc + 1) * o_tiles], scale_cols[:]
                )
                sc_bc = scbc.tile([P, och], F32, tag="scbc")
                sc_slice = scale_dram[oc * och:(oc + 1) * och]
                nc.scalar.dma_start(
                    sc_bc[:],
                    bass.AP(tensor=sc_slice.tensor, offset=sc_slice.offset,
                            ap=[[0, P]] + list(sc_slice.ap)),
                )
                bias_bc = scbc.tile([P, och], F32, tag="biasbc")
                b_slice = b_dram[oc * och:(oc + 1) * och]
                nc.scalar.dma_start(
                    bias_bc[:],
                    bass.AP(tensor=b_slice.tensor, offset=b_slice.offset,
                            ap=[[0, P]] + list(b_slice.ap)),
                )
                return st, sc_bc, bias_bc

            def emit_mm_block(oc, t, st, sc_bc, bias_bc):
                pm = psum_mm.tile([P, och], F32, tag="mmps")
                for i in range(i_tiles):
                    nc.tensor.matmul(
                        pm[:],
                        xt[:, i, t * P:(t + 1) * P],
                        st[:, i, :],
                        start=(i == 0), stop=(i == i_tiles - 1),
                    )
                ob = outsb.tile([P, och], F32, tag="ob")
                nc.vector.tensor_mul(out=ob[:], in0=pm[:], in1=sc_bc[:])
                nc.vector.tensor_add(out=ob[:], in0=ob[:], in1=bias_bc[:])
                nc.gpsimd.dma_start(
                    out_dram[t * P:(t + 1) * P, oc * och:(oc + 1) * och],
                    ob[:],
                )

            # ---- emission order: prologue interleaves X build with W(oc0/1)
            # so the first MM fires ~25-40us in and the PE never starves.
            xl = {}
            xl[0] = emit_x_load(0)
            xl[1] = emit_x_load(1)
            xl[2] = emit_x_load(2)
            emit_x_cast_tr(0, xl[0])
            wctx = {}
            wctx[0] = emit_w_block(0)
            emit_x_cast_tr(1, xl[1])
            xl[3] = emit_x_load(3)
            xl[4] = emit_x_load(4)
            emit_x_cast_tr(2, xl[2])
            emit_x_cast_tr(3, xl[3])
            xl[5] = emit_x_load(5)
            wctx[1] = emit_w_block(1)
            emit_x_cast_tr(4, xl[4])
            xl[6] = emit_x_load(6)
            emit_x_cast_tr(5, xl[5])
            xl[7] = emit_x_load(7)
            emit_x_cast_tr(6, xl[6])
            emit_x_cast_tr(7, xl[7])

            for oc in range(oc_chunks):
                for t in range(t_tiles):
                    emit_mm_block(oc, t, *wctx[oc])
                if oc + 2 < oc_chunks:
                    wctx[oc + 2] = emit_w_block(oc + 2)
                del wctx[oc]

    nc.finalize()
    return nc


_CACHE = {}


def kernel(inputs, weight, bias):
    from concourse.bass_utils import run_bass_kernel_spmd

    x = np.ascontiguousarray(np.asarray(inputs, dtype=np.float32))
    w = np.ascontiguousarray(np.asarray(weight, dtype=np.float32))
    b = np.ascontiguousarray(np.asarray(bias, dtype=np.float32))
    assert x.shape == (TOKENS, IN_FEATURES)
    assert w.shape == (OUT_FEATURES, IN_FEATURES)
    assert b.shape == (OUT_FEATURES,)

    if "nc" not in _CACHE:
        _CACHE["nc"] = build_nc(TOKENS // N_CORES, IN_FEATURES, OUT_FEATURES)
    nc = _CACHE["nc"]

    shards = np.split(x, N_CORES, axis=0)
    in_maps = [{"x": shards[c], "w": w, "b": b} for c in range(N_CORES)]
    trace = bool(os.environ.get("BASS_TRACE"))
    res = run_bass_kernel_spmd(nc, in_maps, list(range(N_CORES)), trace=trace)
    if trace:
        _CACHE["last_result"] = res
        if res.exec_time_ns is not None:
            print(f"HW exec time: {res.exec_time_ns} ns")

    return np.concatenate([res.results[c]["out"] for c in range(N_CORES)], axis=0)
